# revision 19
# baseline (speedup 1.0000x reference)
"""Binarized 1D convolution (K=5, Cin=Cout=256, SAME padding) + bias + ReLU
on 8 Trainium2 NeuronCores, data-parallel over the batch dimension.

Full inputs in, full output out:
  x: [64, 4096, 256] f32, W: [5, 256, 256] f32, b: [256] f32
  out[n, l, co] = relu(b[co] + sum_{k,ci} x[n, l+k-2, ci] * sign(W[k, ci, co]))

Per-core plan (8 batch rows each):
  - PE-transpose x [l, ci] -> strip [ci, l] (128x128 blocks, f32, via identity
    matmul), assembled into a whole-row strip with 2 zero halo columns on each
    side for the SAME-padding edges.
  - Conv as 10 accumulating matmuls per 128-l output tile: lhsT = shifted
    [ci=128, l=128] window of the strip (stationary), rhs = sign(W[k])[ci=128,
    co=256] (moving), accumulated in PSUM over k in 0..4 and both ci halves.
  - Bias folded in as a K=1 matmul (ones[1,128].T @ b[1,256]) that also
    initializes the PSUM accumulation group.
  - ReLU on the scalar engine PSUM -> SBUF, 512-l coalesced stores.
"""

import numpy as np

B, L, CIN, COUT, KW = 64, 4096, 256, 256, 5
N_CORES = 8
B_PER_CORE = B // N_CORES
P = 128

_CACHE = {}


def _build():
    import concourse.bass as bass
    import concourse.mybir as mybir
    import concourse.tile as tile
    from concourse import bacc
    from concourse.masks import make_identity

    f32 = mybir.dt.float32
    f32r = mybir.dt.float32r

    nc = bacc.Bacc("TRN2", target_bir_lowering=False, debug=False)
    x_d = nc.dram_tensor("x", (B_PER_CORE, L, CIN), f32r, kind="ExternalInput")
    w_d = nc.dram_tensor("W", (KW, CIN, COUT), f32, kind="ExternalInput")
    b_d = nc.dram_tensor("b", (1, COUT), f32r, kind="ExternalInput")
    out_d = nc.dram_tensor("out", (B_PER_CORE, L, COUT), f32, kind="ExternalOutput")

    CHUNK = 1024  # l positions per load/store DMA
    NBLK = CHUNK // P  # 128-l blocks per chunk

    with tile.TileContext(nc) as tc:
        with (
            tc.tile_pool(name="const", bufs=1) as const_pool,
            tc.tile_pool(name="wb", bufs=1) as wb_pool,
            tc.tile_pool(name="xc", bufs=4) as xc_pool,
            tc.tile_pool(name="strip", bufs=4) as strip_pool,
            tc.tile_pool(name="ow", bufs=4) as ow_pool,
            tc.tile_pool(name="pt", bufs=3, space=bass.MemorySpace.PSUM) as pt_pool,
            tc.tile_pool(name="po", bufs=4, space=bass.MemorySpace.PSUM) as po_pool,
            tc.tile_pool(name="pb", bufs=1, space=bass.MemorySpace.PSUM) as pb_pool,
        ):
            ident_f32 = const_pool.tile([P, P], f32)
            make_identity(nc, ident_f32[:])
            ident = const_pool.tile([P, P], f32r)
            nc.vector.tensor_copy(ident[:], ident_f32[:])
            ones_f32 = const_pool.tile([1, P], f32)
            nc.gpsimd.memset(ones_f32[:], 1.0)
            ones = const_pool.tile([1, P], f32r)
            nc.vector.tensor_copy(ones[:], ones_f32[:])
            brow = const_pool.tile([1, COUT], f32r)
            nc.sync.dma_start(brow[:], b_d.ap()[:])
            # bias broadcast to all 128 partitions via a one-time K=1 matmul
            bias_bc = const_pool.tile([P, COUT], f32)
            pb = pb_pool.tile([P, COUT], f32, tag="pb")
            nc.tensor.matmul(
                pb[:],
                ones[:],
                brow[:],
                start=True,
                stop=True,
            )
            nc.vector.tensor_copy(bias_bc[:], pb[:])

            # Binarized weights: one DMA for all taps/halves, one sign pass.
            # Layout [ci=128, (k, ciT), co]; slice (k, ciT) per matmul.
            def setup_weights():
                wraw = wb_pool.tile([P, 2 * KW, COUT], f32, tag="wraw")
                nc.sync.dma_start(
                    wraw[:], w_d.ap().rearrange("k (t p) c -> p (k t) c", p=P)
                )
                wb_all = wb_pool.tile([P, 2 * KW, COUT], f32r, tag="wb")
                nc.scalar.sign(wb_all[:], wraw[:])
                return {
                    (k, ciT): wb_all[:, 2 * k + ciT, :]
                    for k in range(KW)
                    for ciT in range(2)
                }

            # Per-chunk strips: [128 ci, 2 ci-halves, clen+6 cols], col j of
            # chunk c0 holds l = c0 - 2 + j. Leading halo (2 cols) comes from
            # the previous strip (memset at row start); trailing halo (2
            # cols) is stitched in by the NEXT chunk's first transposed block
            # (memset at row end).
            SW = CHUNK + 6

            def transpose_chunk(strip, strip_prev, prev_clen, r, c0, clen):
                nblk = clen // P
                xc = xc_pool.tile([P, NBLK, CIN], f32r, tag="xc")
                nc.sync.dma_start(
                    xc[:, :nblk, :],
                    x_d.ap()[r, c0 : c0 + clen, :].rearrange(
                        "(n p) c -> p n c", p=P
                    ),
                )
                if strip_prev is not None:
                    # leading halo (l = c0-2, c0-1) from the previous strip
                    for ciT in range(2):
                        nc.vector.tensor_copy(
                            strip[:, ciT, 0:2],
                            strip_prev[:, ciT, prev_clen : prev_clen + 2],
                        )
                for i in range(nblk):
                    for ciT in range(2):
                        pt = pt_pool.tile([P, P], f32r, tag="pt")
                        nc.tensor.transpose(
                            pt[:],
                            xc[:, i, ciT * P : (ciT + 1) * P],
                            ident[:],
                        )
                        col = 2 + i * P
                        nc.vector.tensor_copy(
                            strip[:, ciT, col : col + P], pt[:]
                        )
                        if i == 0 and strip_prev is not None:
                            nc.vector.tensor_copy(
                                strip_prev[
                                    :, ciT, 2 + prev_clen : 4 + prev_clen
                                ],
                                pt[:, 0:2],
                            )

            def matmul_chunk(strip, r, c0, clen):
                nblk = clen // P
                ow = ow_pool.tile([P, NBLK, COUT], f32, tag="ow")
                for i in range(nblk):
                    po = po_pool.tile([P, COUT], f32, tag="po")
                    for ciT in range(2):
                        for k in range(KW):
                            first = ciT == 0 and k == 0
                            last = ciT == 1 and k == KW - 1
                            nc.tensor.matmul(
                                po[:],
                                strip[:, ciT, i * P + k : i * P + k + P],
                                wb[(k, ciT)],
                                start=first,
                                stop=last,
                            )
                    # bias add on DVE, then ReLU in place on ACT
                    nc.vector.scalar_tensor_tensor(
                        ow[:, i, :],
                        po[:],
                        0.0,
                        bias_bc[:],
                        mybir.AluOpType.add,
                        mybir.AluOpType.add,
                    )
                    nc.scalar.activation(
                        ow[:, i, :], ow[:, i, :], mybir.ActivationFunctionType.Relu
                    )
                nc.sync.dma_start(
                    out_d.ap()[r, c0 : c0 + clen, :].rearrange(
                        "(n p) c -> p n c", p=P
                    ),
                    ow[:, :nblk, :],
                )

            # Global software pipeline over all (row, chunk) pairs:
            # T(n+1) is emitted before MM(n) so MM(n)'s trailing halo
            # (written by T(n+1)'s first block) is already in flight.
            # The final chunk is split small so the kernel tail drains fast.
            chunks = []
            for r in range(B_PER_CORE):
                sizes = [CHUNK] * (L // CHUNK)
                if r == 0:
                    # small leading chunks: fill the pipeline fast at start
                    sizes = [CHUNK // 4, CHUNK // 4, CHUNK // 2] + sizes[1:]
                if r == B_PER_CORE - 1:
                    # small trailing chunks: drain the tail fast at the end
                    sizes = sizes[:-1] + [CHUNK // 2, CHUNK // 4, CHUNK // 4]
                c0 = 0
                for s in sizes:
                    chunks.append((r, c0, s))
                    c0 += s

            def new_strip(r, c0, clen):
                strip = strip_pool.tile([P, 2, SW], f32r, tag="strip")
                if c0 == 0:
                    for ciT in range(2):
                        nc.gpsimd.memset(
                            strip[:, ciT, 0:2].bitcast(mybir.dt.uint32), 0
                        )
                if c0 + clen == L:
                    for ciT in range(2):
                        nc.gpsimd.memset(
                            strip[:, ciT, 2 + clen : 4 + clen].bitcast(
                                mybir.dt.uint32
                            ),
                            0,
                        )
                return strip

            strips = {}
            strips[0] = new_strip(*chunks[0])
            transpose_chunk(strips[0], None, 0, *chunks[0])
            wb = setup_weights()
            for n in range(len(chunks)):
                if n + 1 < len(chunks):
                    rn, cn, sn = chunks[n + 1]
                    strips[n + 1] = new_strip(rn, cn, sn)
                    # same row: stitch trailing halo of chunk n
                    prev = strips[n] if cn != 0 else None
                    transpose_chunk(
                        strips[n + 1], prev, chunks[n][2], rn, cn, sn
                    )
                matmul_chunk(strips[n], *chunks[n])
                del strips[n]

    nc.compile()
    return nc


def _get_nc():
    if "nc" not in _CACHE:
        _CACHE["nc"] = _build()
    return _CACHE["nc"]


def kernel(x: np.ndarray, W: np.ndarray, b: np.ndarray) -> np.ndarray:
    from concourse import bass_utils

    nc = _get_nc()
    x = np.ascontiguousarray(x, dtype=np.float32)
    W = np.ascontiguousarray(W, dtype=np.float32)
    b2 = np.ascontiguousarray(b, dtype=np.float32).reshape(1, COUT)
    in_maps = [
        {
            "x": x[i * B_PER_CORE : (i + 1) * B_PER_CORE],
            "W": W,
            "b": b2,
        }
        for i in range(N_CORES)
    ]
    res = bass_utils.run_bass_kernel_spmd(nc, in_maps, core_ids=list(range(N_CORES)))
    return np.concatenate([res.results[i]["out"] for i in range(N_CORES)], axis=0)


# revision 30
# speedup vs baseline: 1.0086x; 1.0086x over previous
"""Binarized 1D convolution (K=5, Cin=Cout=256, SAME padding) + bias + ReLU
on 8 Trainium2 NeuronCores, data-parallel over the batch dimension.

Full inputs in, full output out:
  x: [64, 4096, 256] f32, W: [5, 256, 256] f32, b: [256] f32
  out[n, l, co] = relu(b[co] + sum_{k,ci} x[n, l+k-2, ci] * sign(W[k, ci, co]))

Per-core plan (8 batch rows each, identical SPMD program):
  - Activations flow through the PE as float32r (full 4-byte storage, PE
    rounds internally; ~1e-4 relative error, 4x faster than plain f32
    matmuls). Binarized weights are exactly representable.
  - Per 1024-l chunk: DMA x[l, ci] naturally, PE-transpose 128x128 blocks
    (identity matmul) into a [ci, l] strip with 2-column halos on each side;
    halos are stitched from the neighboring chunks (zeros at row edges for
    SAME padding).
  - Conv as 10 accumulating matmuls per 128-l output tile: lhsT = shifted
    [ci=128, l=128] strip window (stationary), rhs = sign(W[k]) [ci=128,
    co=256] (moving), PSUM-accumulated over k in 0..4 and both ci halves.
  - Bias is broadcast once via a K=1 matmul into SBUF, added per-tile on the
    vector engine (PSUM -> SBUF), ReLU in place on the scalar engine,
    1 MB coalesced stores.
  - Software pipeline: transposes run one chunk ahead of the matmuls; the
    first/last chunks are split small to fill and drain the pipeline fast.
"""

import numpy as np

B, L, CIN, COUT, KW = 64, 4096, 256, 256, 5
N_CORES = 8
B_PER_CORE = B // N_CORES
P = 128
CHUNK = 1024  # l positions per load/store DMA
LA = 1  # transpose lookahead (chunks emitted ahead of their matmuls)

_CACHE = {}


def _build():
    import concourse.bass as bass
    import concourse.mybir as mybir
    import concourse.tile as tile
    from concourse import bacc
    from concourse.masks import make_identity

    f32 = mybir.dt.float32
    f32r = mybir.dt.float32r
    u32 = mybir.dt.uint32

    nc = bacc.Bacc("TRN2", target_bir_lowering=False, debug=False)
    # x and b are declared float32r: identical bytes/numpy view as float32,
    # but walrus requires f32r matmul operands to be produced as f32r.
    x_d = nc.dram_tensor("x", (B_PER_CORE, L, CIN), f32r, kind="ExternalInput")
    w_d = nc.dram_tensor("W", (KW, CIN, COUT), f32, kind="ExternalInput")
    b_d = nc.dram_tensor("b", (1, COUT), f32r, kind="ExternalInput")
    out_d = nc.dram_tensor("out", (B_PER_CORE, L, COUT), f32, kind="ExternalOutput")

    NBLK = CHUNK // P  # max 128-l blocks per chunk

    with tile.TileContext(nc) as tc:
        with (
            tc.tile_pool(name="const", bufs=1) as const_pool,
            tc.tile_pool(name="wb", bufs=1) as wb_pool,
            tc.tile_pool(name="xc", bufs=6) as xc_pool,
            tc.tile_pool(name="strip", bufs=7) as strip_pool,
            tc.tile_pool(name="ow", bufs=4) as ow_pool,
            tc.tile_pool(name="pt", bufs=3, space=bass.MemorySpace.PSUM) as pt_pool,
            tc.tile_pool(name="po", bufs=4, space=bass.MemorySpace.PSUM) as po_pool,
            tc.tile_pool(name="pb", bufs=1, space=bass.MemorySpace.PSUM) as pb_pool,
        ):
            ident_f32 = const_pool.tile([P, P], f32)
            make_identity(nc, ident_f32[:])
            ident = const_pool.tile([P, P], f32r)
            nc.vector.tensor_copy(ident[:], ident_f32[:])

            def setup_bias():
                ones_f32 = const_pool.tile([1, P], f32)
                nc.gpsimd.memset(ones_f32[:], 1.0)
                ones = const_pool.tile([1, P], f32r)
                nc.vector.tensor_copy(ones[:], ones_f32[:])
                brow = const_pool.tile([1, COUT], f32r)
                nc.sync.dma_start(brow[:], b_d.ap()[:])
                # bias broadcast to all partitions via a one-time K=1 matmul
                bias_bc = const_pool.tile([P, COUT], f32)
                pb = pb_pool.tile([P, COUT], f32, tag="pb")
                nc.tensor.matmul(pb[:], ones[:], brow[:], start=True, stop=True)
                nc.vector.tensor_copy(bias_bc[:], pb[:])
                return bias_bc

            # Binarized weights: layout [ci=128, (k, ciT), co], loaded and
            # sign-binarized in two halves so the first matmuls start sooner.
            wraw = wb_pool.tile([P, 2 * KW, COUT], f32, tag="wraw")
            wsrc = w_d.ap().rearrange("k (t p) c -> p (k t) c", p=P)
            wb_all = wb_pool.tile([P, 2 * KW, COUT], f32r, tag="wb")

            def setup_weights_half(h0, h1):
                nc.sync.dma_start(wraw[:, h0:h1, :], wsrc[:, h0:h1, :])
                nc.scalar.sign(wb_all[:, h0:h1, :], wraw[:, h0:h1, :])

            wb = {
                (k, ciT): wb_all[:, 2 * k + ciT, :]
                for k in range(KW)
                for ciT in range(2)
            }

            # Per-chunk strips: [128 ci, 2 ci-halves, clen+6 cols], col j of
            # chunk c0 holds l = c0 - 2 + j. Leading halo (2 cols) comes from
            # the previous strip (memset at row start); trailing halo (2
            # cols) is stitched in by the NEXT chunk's first transposed block
            # (memset at row end).
            SW = CHUNK + 6

            def transpose_chunk(strip, strip_prev, prev_clen, r, c0, clen):
                nblk = clen // P
                xc = xc_pool.tile([P, NBLK, CIN], f32r, tag="xc")
                nc.sync.dma_start(
                    xc[:, :nblk, :],
                    x_d.ap()[r, c0 : c0 + clen, :].rearrange(
                        "(n p) c -> p n c", p=P
                    ),
                )
                if strip_prev is not None:
                    # leading halo (l = c0-2, c0-1) from the previous strip
                    for ciT in range(2):
                        nc.vector.tensor_copy(
                            strip[:, ciT, 0:2],
                            strip_prev[:, ciT, prev_clen : prev_clen + 2],
                        )
                for i in range(nblk):
                    for ciT in range(2):
                        pt = pt_pool.tile([P, P], f32r, tag="pt")
                        nc.tensor.transpose(
                            pt[:], xc[:, i, ciT * P : (ciT + 1) * P], ident[:]
                        )
                        col = 2 + i * P
                        nc.vector.tensor_copy(
                            strip[:, ciT, col : col + P], pt[:]
                        )
                        if i == 0 and strip_prev is not None:
                            # trailing halo of the previous strip
                            nc.vector.tensor_copy(
                                strip_prev[
                                    :, ciT, 2 + prev_clen : 4 + prev_clen
                                ],
                                pt[:, 0:2],
                            )

            def matmul_chunk(strip, r, c0, clen):
                nblk = clen // P
                ow = ow_pool.tile([P, NBLK, COUT], f32, tag="ow")
                for i in range(nblk):
                    po = po_pool.tile([P, COUT], f32, tag="po")
                    for ciT in range(2):
                        for k in range(KW):
                            nc.tensor.matmul(
                                po[:],
                                strip[:, ciT, i * P + k : i * P + k + P],
                                wb[(k, ciT)],
                                start=(ciT == 0 and k == 0),
                                stop=(ciT == 1 and k == KW - 1),
                            )
                    # bias add on DVE, then ReLU in place on ACT
                    nc.vector.scalar_tensor_tensor(
                        ow[:, i, :],
                        po[:],
                        0.0,
                        bias_bc[:],
                        mybir.AluOpType.add,
                        mybir.AluOpType.add,
                    )
                    nc.scalar.activation(
                        ow[:, i, :], ow[:, i, :], mybir.ActivationFunctionType.Relu
                    )
                nc.sync.dma_start(
                    out_d.ap()[r, c0 : c0 + clen, :].rearrange(
                        "(n p) c -> p n c", p=P
                    ),
                    ow[:, :nblk, :],
                )

            # Chunk list: 1024-l chunks, with the global first/last split
            # small so the pipeline fills and drains quickly.
            chunks = []
            for r in range(B_PER_CORE):
                sizes = [CHUNK] * (L // CHUNK)
                if r == 0:
                    sizes = [CHUNK // 4, CHUNK // 4, CHUNK // 2] + sizes[1:]
                if r == B_PER_CORE - 1:
                    sizes = sizes[:-1] + [CHUNK // 2, CHUNK // 4, CHUNK // 4]
                c0 = 0
                for s in sizes:
                    chunks.append((r, c0, s))
                    c0 += s

            def new_strip(r, c0, clen):
                strip = strip_pool.tile([P, 2, SW], f32r, tag="strip")
                # SAME-padding zeros at row edges (uint32 view: gpsimd
                # memset cannot encode f32r directly)
                if c0 == 0:
                    for ciT in range(2):
                        nc.gpsimd.memset(strip[:, ciT, 0:2].bitcast(u32), 0)
                if c0 + clen == L:
                    for ciT in range(2):
                        nc.gpsimd.memset(
                            strip[:, ciT, 2 + clen : 4 + clen].bitcast(u32), 0
                        )
                return strip

            def emit_transpose(n):
                rn, cn, sn = chunks[n]
                strips[n] = new_strip(rn, cn, sn)
                prev = strips.get(n - 1) if cn != 0 else None
                prev_clen = chunks[n - 1][2] if n > 0 else 0
                transpose_chunk(strips[n], prev, prev_clen, rn, cn, sn)

            strips = {}
            for n in range(min(LA, len(chunks))):
                emit_transpose(n)
            setup_weights_half(0, KW)
            setup_weights_half(KW, 2 * KW)
            bias_bc = setup_bias()
            for n in range(len(chunks)):
                if n + LA < len(chunks):
                    emit_transpose(n + LA)
                matmul_chunk(strips[n], *chunks[n])
                del strips[n]

    nc.compile()
    return nc


def _get_nc():
    if "nc" not in _CACHE:
        _CACHE["nc"] = _build()
    return _CACHE["nc"]


def kernel(x: np.ndarray, W: np.ndarray, b: np.ndarray) -> np.ndarray:
    from concourse import bass_utils

    nc = _get_nc()
    x = np.ascontiguousarray(x, dtype=np.float32)
    W = np.ascontiguousarray(W, dtype=np.float32)
    b2 = np.ascontiguousarray(b, dtype=np.float32).reshape(1, COUT)
    in_maps = [
        {
            "x": x[i * B_PER_CORE : (i + 1) * B_PER_CORE],
            "W": W,
            "b": b2,
        }
        for i in range(N_CORES)
    ]
    res = bass_utils.run_bass_kernel_spmd(nc, in_maps, core_ids=list(range(N_CORES)))
    return np.concatenate([res.results[i]["out"] for i in range(N_CORES)], axis=0)


# revision 31
# speedup vs baseline: 1.0095x; 1.0009x over previous
"""Binarized 1D convolution (K=5, Cin=Cout=256, SAME padding) + bias + ReLU
on 8 Trainium2 NeuronCores, data-parallel over the batch dimension.

Full inputs in, full output out:
  x: [64, 4096, 256] f32, W: [5, 256, 256] f32, b: [256] f32
  out[n, l, co] = relu(b[co] + sum_{k,ci} x[n, l+k-2, ci] * sign(W[k, ci, co]))

Per-core plan (8 batch rows each, identical SPMD program):
  - Activations flow through the PE as float32r (full 4-byte storage, PE
    rounds internally; ~1e-4 relative error, 4x faster than plain f32
    matmuls). Binarized weights are exactly representable.
  - Per 1024-l chunk: DMA x[l, ci] naturally, PE-transpose 128x128 blocks
    (identity matmul) into a [ci, l] strip with 2-column halos on each side;
    halos are stitched from the neighboring chunks (zeros at row edges for
    SAME padding).
  - Conv as 10 accumulating matmuls per 128-l output tile: lhsT = shifted
    [ci=128, l=128] strip window (stationary), rhs = sign(W[k]) [ci=128,
    co=256] (moving), PSUM-accumulated over k in 0..4 and both ci halves.
  - Bias is broadcast once via a K=1 matmul into SBUF, added per-tile on the
    vector engine (PSUM -> SBUF), ReLU in place on the scalar engine,
    1 MB coalesced stores.
  - Software pipeline: transposes run one chunk ahead of the matmuls; the
    first/last chunks are split small to fill and drain the pipeline fast.
"""

import numpy as np

B, L, CIN, COUT, KW = 64, 4096, 256, 256, 5
N_CORES = 8
B_PER_CORE = B // N_CORES
P = 128
CHUNK = 1024  # l positions per load/store DMA
LA = 1  # transpose lookahead (chunks emitted ahead of their matmuls)

_CACHE = {}


def _build():
    import concourse.bass as bass
    import concourse.mybir as mybir
    import concourse.tile as tile
    from concourse import bacc
    from concourse.masks import make_identity

    f32 = mybir.dt.float32
    f32r = mybir.dt.float32r
    u32 = mybir.dt.uint32

    nc = bacc.Bacc("TRN2", target_bir_lowering=False, debug=False)
    # x and b are declared float32r: identical bytes/numpy view as float32,
    # but walrus requires f32r matmul operands to be produced as f32r.
    x_d = nc.dram_tensor("x", (B_PER_CORE, L, CIN), f32r, kind="ExternalInput")
    w_d = nc.dram_tensor("W", (KW, CIN, COUT), f32, kind="ExternalInput")
    b_d = nc.dram_tensor("b", (1, COUT), f32r, kind="ExternalInput")
    out_d = nc.dram_tensor("out", (B_PER_CORE, L, COUT), f32, kind="ExternalOutput")

    NBLK = CHUNK // P  # max 128-l blocks per chunk

    with tile.TileContext(nc) as tc:
        with (
            tc.tile_pool(name="const", bufs=1) as const_pool,
            tc.tile_pool(name="wb", bufs=1) as wb_pool,
            tc.tile_pool(name="xc", bufs=6) as xc_pool,
            tc.tile_pool(name="strip", bufs=7) as strip_pool,
            tc.tile_pool(name="ow", bufs=4) as ow_pool,
            tc.tile_pool(name="pt", bufs=3, space=bass.MemorySpace.PSUM) as pt_pool,
            tc.tile_pool(name="po", bufs=5, space=bass.MemorySpace.PSUM) as po_pool,
        ):
            ident_f32 = const_pool.tile([P, P], f32)
            make_identity(nc, ident_f32[:])
            ident = const_pool.tile([P, P], f32r)
            nc.vector.tensor_copy(ident[:], ident_f32[:])

            def setup_bias():
                ones_f32 = const_pool.tile([1, P], f32)
                nc.gpsimd.memset(ones_f32[:], 1.0)
                ones = const_pool.tile([1, P], f32r)
                nc.vector.tensor_copy(ones[:], ones_f32[:])
                brow = const_pool.tile([1, COUT], f32r)
                nc.sync.dma_start(brow[:], b_d.ap()[:])
                # bias broadcast to all partitions via a one-time K=1 matmul
                bias_bc = const_pool.tile([P, COUT], f32)
                pb = pt_pool.tile([P, COUT], f32, tag="pt")
                nc.tensor.matmul(pb[:], ones[:], brow[:], start=True, stop=True)
                nc.vector.tensor_copy(bias_bc[:], pb[:])
                return bias_bc

            # Binarized weights: layout [ci=128, (k, ciT), co], loaded and
            # sign-binarized in two halves so the first matmuls start sooner.
            wraw = wb_pool.tile([P, 2 * KW, COUT], f32, tag="wraw")
            wsrc = w_d.ap().rearrange("k (t p) c -> p (k t) c", p=P)
            wb_all = wb_pool.tile([P, 2 * KW, COUT], f32r, tag="wb")

            def setup_weights_half(h0, h1):
                nc.sync.dma_start(wraw[:, h0:h1, :], wsrc[:, h0:h1, :])
                nc.scalar.sign(wb_all[:, h0:h1, :], wraw[:, h0:h1, :])

            wb = {
                (k, ciT): wb_all[:, 2 * k + ciT, :]
                for k in range(KW)
                for ciT in range(2)
            }

            # Per-chunk strips: [128 ci, 2 ci-halves, clen+6 cols], col j of
            # chunk c0 holds l = c0 - 2 + j. Leading halo (2 cols) comes from
            # the previous strip (memset at row start); trailing halo (2
            # cols) is stitched in by the NEXT chunk's first transposed block
            # (memset at row end).
            SW = CHUNK + 6

            def transpose_chunk(strip, strip_prev, prev_clen, r, c0, clen):
                nblk = clen // P
                xc = xc_pool.tile([P, NBLK, CIN], f32r, tag="xc")
                nc.sync.dma_start(
                    xc[:, :nblk, :],
                    x_d.ap()[r, c0 : c0 + clen, :].rearrange(
                        "(n p) c -> p n c", p=P
                    ),
                )
                if strip_prev is not None:
                    # leading halo (l = c0-2, c0-1) from the previous strip
                    for ciT in range(2):
                        nc.vector.tensor_copy(
                            strip[:, ciT, 0:2],
                            strip_prev[:, ciT, prev_clen : prev_clen + 2],
                        )
                for i in range(nblk):
                    for ciT in range(2):
                        pt = pt_pool.tile([P, P], f32r, tag="pt")
                        nc.tensor.transpose(
                            pt[:], xc[:, i, ciT * P : (ciT + 1) * P], ident[:]
                        )
                        col = 2 + i * P
                        nc.vector.tensor_copy(
                            strip[:, ciT, col : col + P], pt[:]
                        )
                        if i == 0 and strip_prev is not None:
                            # trailing halo of the previous strip
                            nc.vector.tensor_copy(
                                strip_prev[
                                    :, ciT, 2 + prev_clen : 4 + prev_clen
                                ],
                                pt[:, 0:2],
                            )

            def matmul_chunk(strip, r, c0, clen):
                nblk = clen // P
                ow = ow_pool.tile([P, NBLK, COUT], f32, tag="ow")
                for i in range(nblk):
                    po = po_pool.tile([P, COUT], f32, tag="po")
                    for ciT in range(2):
                        for k in range(KW):
                            nc.tensor.matmul(
                                po[:],
                                strip[:, ciT, i * P + k : i * P + k + P],
                                wb[(k, ciT)],
                                start=(ciT == 0 and k == 0),
                                stop=(ciT == 1 and k == KW - 1),
                            )
                    # bias add on DVE, then ReLU in place on ACT
                    nc.vector.scalar_tensor_tensor(
                        ow[:, i, :],
                        po[:],
                        0.0,
                        bias_bc[:],
                        mybir.AluOpType.add,
                        mybir.AluOpType.add,
                    )
                    nc.scalar.activation(
                        ow[:, i, :], ow[:, i, :], mybir.ActivationFunctionType.Relu
                    )
                nc.sync.dma_start(
                    out_d.ap()[r, c0 : c0 + clen, :].rearrange(
                        "(n p) c -> p n c", p=P
                    ),
                    ow[:, :nblk, :],
                )

            # Chunk list: 1024-l chunks, with the global first/last split
            # small so the pipeline fills and drains quickly.
            chunks = []
            for r in range(B_PER_CORE):
                sizes = [CHUNK] * (L // CHUNK)
                if r == 0:
                    sizes = [CHUNK // 4, CHUNK // 4, CHUNK // 2] + sizes[1:]
                if r == B_PER_CORE - 1:
                    sizes = sizes[:-1] + [CHUNK // 2, CHUNK // 4, CHUNK // 8, CHUNK // 8]
                c0 = 0
                for s in sizes:
                    chunks.append((r, c0, s))
                    c0 += s

            def new_strip(r, c0, clen):
                strip = strip_pool.tile([P, 2, SW], f32r, tag="strip")
                # SAME-padding zeros at row edges (uint32 view: gpsimd
                # memset cannot encode f32r directly)
                if c0 == 0:
                    for ciT in range(2):
                        nc.gpsimd.memset(strip[:, ciT, 0:2].bitcast(u32), 0)
                if c0 + clen == L:
                    for ciT in range(2):
                        nc.gpsimd.memset(
                            strip[:, ciT, 2 + clen : 4 + clen].bitcast(u32), 0
                        )
                return strip

            def emit_transpose(n):
                rn, cn, sn = chunks[n]
                strips[n] = new_strip(rn, cn, sn)
                prev = strips.get(n - 1) if cn != 0 else None
                prev_clen = chunks[n - 1][2] if n > 0 else 0
                transpose_chunk(strips[n], prev, prev_clen, rn, cn, sn)

            strips = {}
            for n in range(min(LA, len(chunks))):
                emit_transpose(n)
            setup_weights_half(0, KW)
            setup_weights_half(KW, 2 * KW)
            bias_bc = setup_bias()
            for n in range(len(chunks)):
                if n + LA < len(chunks):
                    emit_transpose(n + LA)
                matmul_chunk(strips[n], *chunks[n])
                del strips[n]

    nc.compile()
    return nc


def _get_nc():
    if "nc" not in _CACHE:
        _CACHE["nc"] = _build()
    return _CACHE["nc"]


def kernel(x: np.ndarray, W: np.ndarray, b: np.ndarray) -> np.ndarray:
    from concourse import bass_utils

    nc = _get_nc()
    x = np.ascontiguousarray(x, dtype=np.float32)
    W = np.ascontiguousarray(W, dtype=np.float32)
    b2 = np.ascontiguousarray(b, dtype=np.float32).reshape(1, COUT)
    in_maps = [
        {
            "x": x[i * B_PER_CORE : (i + 1) * B_PER_CORE],
            "W": W,
            "b": b2,
        }
        for i in range(N_CORES)
    ]
    res = bass_utils.run_bass_kernel_spmd(nc, in_maps, core_ids=list(range(N_CORES)))
    return np.concatenate([res.results[i]["out"] for i in range(N_CORES)], axis=0)


# revision 34
# speedup vs baseline: 1.0102x; 1.0007x over previous
"""Binarized 1D convolution (K=5, Cin=Cout=256, SAME padding) + bias + ReLU
on 8 Trainium2 NeuronCores, data-parallel over the batch dimension.

Full inputs in, full output out:
  x: [64, 4096, 256] f32, W: [5, 256, 256] f32, b: [256] f32
  out[n, l, co] = relu(b[co] + sum_{k,ci} x[n, l+k-2, ci] * sign(W[k, ci, co]))

Per-core plan (8 batch rows each, identical SPMD program):
  - Activations flow through the PE as float32r (full 4-byte storage, PE
    rounds internally; ~1e-4 relative error, 4x faster than plain f32
    matmuls). Binarized weights are exactly representable.
  - Per 1024-l chunk: DMA x[l, ci] naturally, PE-transpose 128x128 blocks
    (identity matmul) into a [ci, l] strip with 2-column halos on each side;
    halos are stitched from the neighboring chunks (zeros at row edges for
    SAME padding).
  - Conv as 10 accumulating matmuls per 128-l output tile: lhsT = shifted
    [ci=128, l=128] strip window (stationary), rhs = sign(W[k]) [ci=128,
    co=256] (moving), PSUM-accumulated over k in 0..4 and both ci halves.
  - Bias is broadcast once via a K=1 matmul into SBUF, added per-tile on the
    vector engine (PSUM -> SBUF), ReLU in place on the scalar engine,
    1 MB coalesced stores.
  - Software pipeline: transposes run one chunk ahead of the matmuls; the
    first/last chunks are split small to fill and drain the pipeline fast.
"""

import numpy as np

B, L, CIN, COUT, KW = 64, 4096, 256, 256, 5
N_CORES = 8
B_PER_CORE = B // N_CORES
P = 128
import os as _os
CHUNK = int(_os.environ.get('KCHUNK', '1024'))  # l per load/store DMA
LA = 1  # transpose lookahead (chunks emitted ahead of their matmuls)

_CACHE = {}


def _build():
    import concourse.bass as bass
    import concourse.mybir as mybir
    import concourse.tile as tile
    from concourse import bacc
    from concourse.masks import make_identity

    f32 = mybir.dt.float32
    f32r = mybir.dt.float32r
    u32 = mybir.dt.uint32

    nc = bacc.Bacc("TRN2", target_bir_lowering=False, debug=False)
    # x and b are declared float32r: identical bytes/numpy view as float32,
    # but walrus requires f32r matmul operands to be produced as f32r.
    x_d = nc.dram_tensor("x", (B_PER_CORE, L, CIN), f32r, kind="ExternalInput")
    w_d = nc.dram_tensor("W", (KW, CIN, COUT), f32, kind="ExternalInput")
    b_d = nc.dram_tensor("b", (1, COUT), f32r, kind="ExternalInput")
    out_d = nc.dram_tensor("out", (B_PER_CORE, L, COUT), f32, kind="ExternalOutput")

    NBLK = CHUNK // P  # max 128-l blocks per chunk

    with tile.TileContext(nc) as tc:
        with (
            tc.tile_pool(name="const", bufs=1) as const_pool,
            tc.tile_pool(name="wb", bufs=1) as wb_pool,
            tc.tile_pool(name="xc", bufs=int(_os.environ.get("KXC", "6"))) as xc_pool,
            tc.tile_pool(name="strip", bufs=int(_os.environ.get("KSTRIP", "7"))) as strip_pool,
            tc.tile_pool(name="ow", bufs=int(_os.environ.get("KOW", "4"))) as ow_pool,
            tc.tile_pool(name="pt", bufs=int(_os.environ.get("KPT", "3")), space=bass.MemorySpace.PSUM) as pt_pool,
            tc.tile_pool(name="po", bufs=int(_os.environ.get("KPO", "5")), space=bass.MemorySpace.PSUM) as po_pool,
        ):
            ident_f32 = const_pool.tile([P, P], f32)
            make_identity(nc, ident_f32[:])
            ident = const_pool.tile([P, P], f32r)
            nc.vector.tensor_copy(ident[:], ident_f32[:])

            def setup_bias():
                ones_f32 = const_pool.tile([1, P], f32)
                nc.gpsimd.memset(ones_f32[:], 1.0)
                ones = const_pool.tile([1, P], f32r)
                nc.vector.tensor_copy(ones[:], ones_f32[:])
                brow = const_pool.tile([1, COUT], f32r)
                nc.sync.dma_start(brow[:], b_d.ap()[:])
                # bias broadcast to all partitions via a one-time K=1 matmul
                bias_bc = const_pool.tile([P, COUT], f32)
                pb = pt_pool.tile([P, COUT], f32, tag="pt")
                nc.tensor.matmul(pb[:], ones[:], brow[:], start=True, stop=True)
                nc.vector.tensor_copy(bias_bc[:], pb[:])
                return bias_bc

            # Binarized weights: layout [ci=128, (k, ciT), co], loaded and
            # sign-binarized in two halves so the first matmuls start sooner.
            wraw = wb_pool.tile([P, 2 * KW, COUT], f32, tag="wraw")
            wsrc = w_d.ap().rearrange("k (t p) c -> p (k t) c", p=P)
            wb_all = wb_pool.tile([P, 2 * KW, COUT], f32r, tag="wb")

            def setup_weights_half(h0, h1):
                nc.sync.dma_start(wraw[:, h0:h1, :], wsrc[:, h0:h1, :])
                nc.scalar.sign(wb_all[:, h0:h1, :], wraw[:, h0:h1, :])

            wb = {
                (k, ciT): wb_all[:, 2 * k + ciT, :]
                for k in range(KW)
                for ciT in range(2)
            }

            # Per-chunk strips: [128 ci, 2 ci-halves, clen+6 cols], col j of
            # chunk c0 holds l = c0 - 2 + j. Leading halo (2 cols) comes from
            # the previous strip (memset at row start); trailing halo (2
            # cols) is stitched in by the NEXT chunk's first transposed block
            # (memset at row end).
            SW = CHUNK + 6

            def transpose_chunk(strip, strip_prev, prev_clen, r, c0, clen):
                nblk = clen // P
                xc = xc_pool.tile([P, NBLK, CIN], f32r, tag="xc")
                nc.sync.dma_start(
                    xc[:, :nblk, :],
                    x_d.ap()[r, c0 : c0 + clen, :].rearrange(
                        "(n p) c -> p n c", p=P
                    ),
                )
                if strip_prev is not None:
                    # leading halo (l = c0-2, c0-1) from the previous strip
                    for ciT in range(2):
                        nc.vector.tensor_copy(
                            strip[:, ciT, 0:2],
                            strip_prev[:, ciT, prev_clen : prev_clen + 2],
                        )
                for i in range(nblk):
                    for ciT in range(2):
                        pt = pt_pool.tile([P, P], f32r, tag="pt")
                        nc.tensor.transpose(
                            pt[:], xc[:, i, ciT * P : (ciT + 1) * P], ident[:]
                        )
                        col = 2 + i * P
                        nc.vector.tensor_copy(
                            strip[:, ciT, col : col + P], pt[:]
                        )
                        if i == 0 and strip_prev is not None:
                            # trailing halo of the previous strip
                            nc.vector.tensor_copy(
                                strip_prev[
                                    :, ciT, 2 + prev_clen : 4 + prev_clen
                                ],
                                pt[:, 0:2],
                            )

            def matmul_chunk(strip, r, c0, clen):
                nblk = clen // P
                ow = ow_pool.tile([P, NBLK, COUT], f32, tag="ow")
                for i in range(nblk):
                    po = po_pool.tile([P, COUT], f32, tag="po")
                    # accumulate in wb-slice order: the first half only needs
                    # the first W-load+sign half, so startup matmuls begin
                    # before the second half lands
                    for idx in range(2 * KW):
                        k, ciT = idx // 2, idx % 2
                        nc.tensor.matmul(
                            po[:],
                            strip[:, ciT, i * P + k : i * P + k + P],
                            wb[(k, ciT)],
                            start=(idx == 0),
                            stop=(idx == 2 * KW - 1),
                        )
                    # bias add on DVE, then ReLU in place on ACT
                    nc.vector.scalar_tensor_tensor(
                        ow[:, i, :],
                        po[:],
                        0.0,
                        bias_bc[:],
                        mybir.AluOpType.add,
                        mybir.AluOpType.add,
                    )
                    nc.scalar.activation(
                        ow[:, i, :], ow[:, i, :], mybir.ActivationFunctionType.Relu
                    )
                nc.sync.dma_start(
                    out_d.ap()[r, c0 : c0 + clen, :].rearrange(
                        "(n p) c -> p n c", p=P
                    ),
                    ow[:, :nblk, :],
                )

            # Chunk list: 1024-l chunks, with the global first/last split
            # small so the pipeline fills and drains quickly.
            chunks = []
            for r in range(B_PER_CORE):
                sizes = [CHUNK] * (L // CHUNK)
                if r == 0:
                    sizes = [CHUNK // 4, CHUNK // 4, CHUNK // 2] + sizes[1:]
                if r == B_PER_CORE - 1:
                    sizes = sizes[:-1] + [CHUNK // 2, CHUNK // 4, CHUNK // 8, CHUNK // 8]
                c0 = 0
                for s in sizes:
                    chunks.append((r, c0, s))
                    c0 += s

            def new_strip(r, c0, clen):
                strip = strip_pool.tile([P, 2, SW], f32r, tag="strip")
                # SAME-padding zeros at row edges (uint32 view: gpsimd
                # memset cannot encode f32r directly)
                if c0 == 0:
                    for ciT in range(2):
                        nc.gpsimd.memset(strip[:, ciT, 0:2].bitcast(u32), 0)
                if c0 + clen == L:
                    for ciT in range(2):
                        nc.gpsimd.memset(
                            strip[:, ciT, 2 + clen : 4 + clen].bitcast(u32), 0
                        )
                return strip

            def emit_transpose(n):
                rn, cn, sn = chunks[n]
                strips[n] = new_strip(rn, cn, sn)
                prev = strips.get(n - 1) if cn != 0 else None
                prev_clen = chunks[n - 1][2] if n > 0 else 0
                transpose_chunk(strips[n], prev, prev_clen, rn, cn, sn)

            strips = {}
            for n in range(min(LA, len(chunks))):
                emit_transpose(n)
            setup_weights_half(0, KW)
            setup_weights_half(KW, 2 * KW)
            bias_bc = setup_bias()
            for n in range(len(chunks)):
                if n + LA < len(chunks):
                    emit_transpose(n + LA)
                matmul_chunk(strips[n], *chunks[n])
                del strips[n]

    nc.compile()
    return nc


def _get_nc():
    if "nc" not in _CACHE:
        _CACHE["nc"] = _build()
    return _CACHE["nc"]


def kernel(x: np.ndarray, W: np.ndarray, b: np.ndarray) -> np.ndarray:
    from concourse import bass_utils

    nc = _get_nc()
    x = np.ascontiguousarray(x, dtype=np.float32)
    W = np.ascontiguousarray(W, dtype=np.float32)
    b2 = np.ascontiguousarray(b, dtype=np.float32).reshape(1, COUT)
    in_maps = [
        {
            "x": x[i * B_PER_CORE : (i + 1) * B_PER_CORE],
            "W": W,
            "b": b2,
        }
        for i in range(N_CORES)
    ]
    res = bass_utils.run_bass_kernel_spmd(nc, in_maps, core_ids=list(range(N_CORES)))
    return np.concatenate([res.results[i]["out"] for i in range(N_CORES)], axis=0)


# revision 35
# speedup vs baseline: 1.0104x; 1.0001x over previous
"""Binarized 1D convolution (K=5, Cin=Cout=256, SAME padding) + bias + ReLU
on 8 Trainium2 NeuronCores, data-parallel over the batch dimension.

Full inputs in, full output out:
  x: [64, 4096, 256] f32, W: [5, 256, 256] f32, b: [256] f32
  out[n, l, co] = relu(b[co] + sum_{k,ci} x[n, l+k-2, ci] * sign(W[k, ci, co]))

Per-core plan (8 batch rows each, identical SPMD program):
  - Activations flow through the PE as float32r (full 4-byte storage, PE
    rounds internally; ~1e-4 relative error, 4x faster than plain f32
    matmuls). Binarized weights are exactly representable.
  - Per 1024-l chunk: DMA x[l, ci] naturally, PE-transpose 128x128 blocks
    (identity matmul) into a [ci, l] strip with 2-column halos on each side;
    halos are stitched from the neighboring chunks (zeros at row edges for
    SAME padding).
  - Conv as 10 accumulating matmuls per 128-l output tile: lhsT = shifted
    [ci=128, l=128] strip window (stationary), rhs = sign(W[k]) [ci=128,
    co=256] (moving), PSUM-accumulated over k in 0..4 and both ci halves.
  - Bias is broadcast once via a K=1 matmul into SBUF, added per-tile on the
    vector engine (PSUM -> SBUF), ReLU in place on the scalar engine,
    1 MB coalesced stores.
  - Software pipeline: transposes run one chunk ahead of the matmuls; the
    first/last chunks are split small to fill and drain the pipeline fast.
"""

import numpy as np

B, L, CIN, COUT, KW = 64, 4096, 256, 256, 5
N_CORES = 8
B_PER_CORE = B // N_CORES
P = 128
import os as _os
CHUNK = int(_os.environ.get('KCHUNK', '1024'))  # l per load/store DMA
LA = 1  # transpose lookahead (chunks emitted ahead of their matmuls)

_CACHE = {}


def _build():
    import concourse.bass as bass
    import concourse.mybir as mybir
    import concourse.tile as tile
    from concourse import bacc
    from concourse.masks import make_identity

    f32 = mybir.dt.float32
    f32r = mybir.dt.float32r
    u32 = mybir.dt.uint32

    nc = bacc.Bacc("TRN2", target_bir_lowering=False, debug=False)
    # x and b are declared float32r: identical bytes/numpy view as float32,
    # but walrus requires f32r matmul operands to be produced as f32r.
    x_d = nc.dram_tensor("x", (B_PER_CORE, L, CIN), f32r, kind="ExternalInput")
    w_d = nc.dram_tensor("W", (KW, CIN, COUT), f32, kind="ExternalInput")
    b_d = nc.dram_tensor("b", (1, COUT), f32r, kind="ExternalInput")
    out_d = nc.dram_tensor("out", (B_PER_CORE, L, COUT), f32, kind="ExternalOutput")

    NBLK = CHUNK // P  # max 128-l blocks per chunk

    with tile.TileContext(nc) as tc:
        with (
            tc.tile_pool(name="const", bufs=1) as const_pool,
            tc.tile_pool(name="wb", bufs=1) as wb_pool,
            tc.tile_pool(name="xc", bufs=int(_os.environ.get("KXC", "6"))) as xc_pool,
            tc.tile_pool(name="strip", bufs=int(_os.environ.get("KSTRIP", "7"))) as strip_pool,
            tc.tile_pool(name="ow", bufs=int(_os.environ.get("KOW", "4"))) as ow_pool,
            tc.tile_pool(name="pt", bufs=int(_os.environ.get("KPT", "3")), space=bass.MemorySpace.PSUM) as pt_pool,
            tc.tile_pool(name="po", bufs=int(_os.environ.get("KPO", "5")), space=bass.MemorySpace.PSUM) as po_pool,
        ):
            ident_f32 = const_pool.tile([P, P], f32)
            make_identity(nc, ident_f32[:])
            ident = const_pool.tile([P, P], f32r)
            nc.vector.tensor_copy(ident[:], ident_f32[:])

            def setup_bias():
                ones_f32 = const_pool.tile([1, P], f32)
                nc.gpsimd.memset(ones_f32[:], 1.0)
                ones = const_pool.tile([1, P], f32r)
                nc.vector.tensor_copy(ones[:], ones_f32[:])
                brow = const_pool.tile([1, COUT], f32r)
                nc.sync.dma_start(brow[:], b_d.ap()[:])
                # bias broadcast to all partitions via a one-time K=1 matmul
                bias_bc = const_pool.tile([P, COUT], f32)
                pb = pt_pool.tile([P, COUT], f32, tag="pt")
                nc.tensor.matmul(pb[:], ones[:], brow[:], start=True, stop=True)
                nc.vector.tensor_copy(bias_bc[:], pb[:])
                return bias_bc, ones, brow

            # Binarized weights: layout [ci=128, (k, ciT), co], loaded and
            # sign-binarized in two halves so the first matmuls start sooner.
            wraw = wb_pool.tile([P, 2 * KW, COUT], f32, tag="wraw")
            wsrc = w_d.ap().rearrange("k (t p) c -> p (k t) c", p=P)
            wb_all = wb_pool.tile([P, 2 * KW, COUT], f32r, tag="wb")

            def setup_weights_half(h0, h1):
                nc.sync.dma_start(wraw[:, h0:h1, :], wsrc[:, h0:h1, :])
                nc.scalar.sign(wb_all[:, h0:h1, :], wraw[:, h0:h1, :])

            wb = {
                (k, ciT): wb_all[:, 2 * k + ciT, :]
                for k in range(KW)
                for ciT in range(2)
            }

            # Per-chunk strips: [128 ci, 2 ci-halves, clen+6 cols], col j of
            # chunk c0 holds l = c0 - 2 + j. Leading halo (2 cols) comes from
            # the previous strip (memset at row start); trailing halo (2
            # cols) is stitched in by the NEXT chunk's first transposed block
            # (memset at row end).
            SW = CHUNK + 6

            def transpose_chunk(strip, strip_prev, prev_clen, r, c0, clen):
                nblk = clen // P
                xc = xc_pool.tile([P, NBLK, CIN], f32r, tag="xc")
                nc.sync.dma_start(
                    xc[:, :nblk, :],
                    x_d.ap()[r, c0 : c0 + clen, :].rearrange(
                        "(n p) c -> p n c", p=P
                    ),
                )
                if strip_prev is not None:
                    # leading halo (l = c0-2, c0-1) from the previous strip
                    for ciT in range(2):
                        nc.vector.tensor_copy(
                            strip[:, ciT, 0:2],
                            strip_prev[:, ciT, prev_clen : prev_clen + 2],
                        )
                for i in range(nblk):
                    for ciT in range(2):
                        pt = pt_pool.tile([P, P], f32r, tag="pt")
                        nc.tensor.transpose(
                            pt[:], xc[:, i, ciT * P : (ciT + 1) * P], ident[:]
                        )
                        col = 2 + i * P
                        nc.vector.tensor_copy(
                            strip[:, ciT, col : col + P], pt[:]
                        )
                        if i == 0 and strip_prev is not None:
                            # trailing halo of the previous strip
                            nc.vector.tensor_copy(
                                strip_prev[
                                    :, ciT, 2 + prev_clen : 4 + prev_clen
                                ],
                                pt[:, 0:2],
                            )

            def matmul_chunk(strip, r, c0, clen, last_chunk=False):
                nblk = clen // P
                ow = ow_pool.tile([P, NBLK, COUT], f32, tag="ow")
                for i in range(nblk):
                    po = po_pool.tile([P, COUT], f32, tag="po")
                    # tail variant: bias via K=1 PE matmul so ACT can ReLU
                    # straight from PSUM -- skips the DVE hop in the drain
                    pe_bias = last_chunk and i == nblk - 1
                    if pe_bias:
                        nc.tensor.matmul(
                            po[:], ones_r[:], brow_r[:], start=True, stop=False
                        )
                    # accumulate in wb-slice order: the first half only needs
                    # the first W-load+sign half, so startup matmuls begin
                    # before the second half lands
                    for idx in range(2 * KW):
                        k, ciT = idx // 2, idx % 2
                        nc.tensor.matmul(
                            po[:],
                            strip[:, ciT, i * P + k : i * P + k + P],
                            wb[(k, ciT)],
                            start=(idx == 0 and not pe_bias),
                            stop=(idx == 2 * KW - 1),
                        )
                    if pe_bias:
                        nc.scalar.activation(
                            ow[:, i, :], po[:], mybir.ActivationFunctionType.Relu
                        )
                    else:
                        # bias add on DVE, then ReLU in place on ACT
                        nc.vector.scalar_tensor_tensor(
                            ow[:, i, :],
                            po[:],
                            0.0,
                            bias_bc[:],
                            mybir.AluOpType.add,
                            mybir.AluOpType.add,
                        )
                        nc.scalar.activation(
                            ow[:, i, :],
                            ow[:, i, :],
                            mybir.ActivationFunctionType.Relu,
                        )
                nc.sync.dma_start(
                    out_d.ap()[r, c0 : c0 + clen, :].rearrange(
                        "(n p) c -> p n c", p=P
                    ),
                    ow[:, :nblk, :],
                )

            # Chunk list: 1024-l chunks, with the global first/last split
            # small so the pipeline fills and drains quickly.
            chunks = []
            for r in range(B_PER_CORE):
                sizes = [CHUNK] * (L // CHUNK)
                if r == 0:
                    sizes = [CHUNK // 4, CHUNK // 4, CHUNK // 2] + sizes[1:]
                if r == B_PER_CORE - 1:
                    sizes = sizes[:-1] + [CHUNK // 2, CHUNK // 4, CHUNK // 8, CHUNK // 8]
                c0 = 0
                for s in sizes:
                    chunks.append((r, c0, s))
                    c0 += s

            def new_strip(r, c0, clen):
                strip = strip_pool.tile([P, 2, SW], f32r, tag="strip")
                # SAME-padding zeros at row edges (uint32 view: gpsimd
                # memset cannot encode f32r directly)
                if c0 == 0:
                    for ciT in range(2):
                        nc.gpsimd.memset(strip[:, ciT, 0:2].bitcast(u32), 0)
                if c0 + clen == L:
                    for ciT in range(2):
                        nc.gpsimd.memset(
                            strip[:, ciT, 2 + clen : 4 + clen].bitcast(u32), 0
                        )
                return strip

            def emit_transpose(n):
                rn, cn, sn = chunks[n]
                strips[n] = new_strip(rn, cn, sn)
                prev = strips.get(n - 1) if cn != 0 else None
                prev_clen = chunks[n - 1][2] if n > 0 else 0
                transpose_chunk(strips[n], prev, prev_clen, rn, cn, sn)

            strips = {}
            for n in range(min(LA, len(chunks))):
                emit_transpose(n)
            setup_weights_half(0, KW)
            setup_weights_half(KW, 2 * KW)
            bias_bc, ones_r, brow_r = setup_bias()
            for n in range(len(chunks)):
                if n + LA < len(chunks):
                    emit_transpose(n + LA)
                matmul_chunk(
                    strips[n], *chunks[n], last_chunk=(n == len(chunks) - 1)
                )
                del strips[n]

    nc.compile()
    return nc


def _get_nc():
    if "nc" not in _CACHE:
        _CACHE["nc"] = _build()
    return _CACHE["nc"]


def kernel(x: np.ndarray, W: np.ndarray, b: np.ndarray) -> np.ndarray:
    from concourse import bass_utils

    nc = _get_nc()
    x = np.ascontiguousarray(x, dtype=np.float32)
    W = np.ascontiguousarray(W, dtype=np.float32)
    b2 = np.ascontiguousarray(b, dtype=np.float32).reshape(1, COUT)
    in_maps = [
        {
            "x": x[i * B_PER_CORE : (i + 1) * B_PER_CORE],
            "W": W,
            "b": b2,
        }
        for i in range(N_CORES)
    ]
    res = bass_utils.run_bass_kernel_spmd(nc, in_maps, core_ids=list(range(N_CORES)))
    return np.concatenate([res.results[i]["out"] for i in range(N_CORES)], axis=0)
